# revision 1
# baseline (speedup 1.0000x reference)
"""NSA (native sparse attention) forward kernel for Trainium2, 8 NeuronCores.

Sharding: tensor-parallel over query heads. Core c computes the full sequence
for heads {2c, 2c+1}. The compressed-attention probabilities (needed by every
core to form the shared per-token top-k block selection) are recomputed on
each core; everything else is computed once.

SPMD: one Bass program; per-core behaviour differs only through input data
(q_own / cw_own slices).

Layout convention on device: scores are computed transposed ([key, query])
so that softmax numerators feed the PV matmul as the stationary operand with
no on-chip transposes; the softmax denominator falls out of an appended
ones-column on the PV moving operand.
"""

import numpy as np

import concourse.bacc as bacc
import concourse.bass as bass
import concourse.tile as tile
from concourse import mybir
from concourse.bass_utils import run_bass_kernel_spmd

# ---- problem constants (hardcoded per contest rules) ----
T, HQ, D = 2048, 16, 128
KS, ST, BS = 32, 16, 64
TOP_N, NINIT, NLOCAL, WIN = 16, 2, 1, 512
M = (T - KS) // ST + 1          # 127 compressed tokens
NB = T // BS                    # 32 selection blocks
NT = T // 128                   # 16 query tiles
NCORES = 8
HPC = HQ // NCORES              # 2 heads per core
SCALE = float(D) ** -0.5
NEGC = -1e30
BIGC = 1e30
EPS = 1e-30
ZAP = -1e38
F32 = mybir.dt.float32

WCH = NT // (WIN // 128)        # swa window spans 4 full chunks + diag


def _build_program():
    nc = bacc.Bacc("TRN2", target_bir_lowering=False, debug=False,
                   num_devices=NCORES)

    d = {}
    d["qt_all"] = nc.dram_tensor("qt_all", [NT, 128, HQ, 128], F32, kind="ExternalInput").ap()
    d["q_own"] = nc.dram_tensor("q_own", [NT, 128, HPC, 128], F32, kind="ExternalInput").ap()
    d["kT"] = nc.dram_tensor("kT", [128, T], F32, kind="ExternalInput").ap()
    d["v_ext"] = nc.dram_tensor("v_ext", [128, NT, 129], F32, kind="ExternalInput").ap()
    d["cmp_kT"] = nc.dram_tensor("cmp_kT", [128, M], F32, kind="ExternalInput").ap()
    d["cmp_rhs"] = nc.dram_tensor("cmp_rhs", [M, 161], F32, kind="ExternalInput").ap()
    d["cmp_keepT"] = nc.dram_tensor("cmp_keepT", [M, NT, 128], F32, kind="ExternalInput").ap()
    d["slc_keep"] = nc.dram_tensor("slc_keep", [128, NT, NB], F32, kind="ExternalInput").ap()
    d["slc_ovr"] = nc.dram_tensor("slc_ovr", [128, NT, NB], F32, kind="ExternalInput").ap()
    d["tri_i"] = nc.dram_tensor("tri_i", [128, 128], F32, kind="ExternalInput").ap()
    d["tri_s"] = nc.dram_tensor("tri_s", [128, 128], F32, kind="ExternalInput").ap()
    d["ident"] = nc.dram_tensor("ident", [128, 128], F32, kind="ExternalInput").ap()
    d["cw_own"] = nc.dram_tensor("cw_own", [128, NT, 3 * HPC], F32, kind="ExternalInput").ap()
    out_ap = nc.dram_tensor("out", [T, HPC, 128], F32, kind="ExternalOutput").ap()

    from contextlib import ExitStack
    with tile.TileContext(nc) as tc:
        with ExitStack() as ctx:
            _body(tc, nc, d, out_ap, ctx)
    nc.compile()
    return nc


def _body(tc, nc, d, out_ap, ctx):
    EXP = mybir.ActivationFunctionType.Exp
    SIG = mybir.ActivationFunctionType.Sigmoid
    CPY = mybir.ActivationFunctionType.Copy
    MUL = mybir.AluOpType.mult
    ADD = mybir.AluOpType.add
    ISEQ = mybir.AluOpType.is_equal

    const = ctx.enter_context(tc.tile_pool(name="const", bufs=1))
    qt_pool = ctx.enter_context(tc.tile_pool(name="qt", bufs=2))
    qo_pool = ctx.enter_context(tc.tile_pool(name="qo", bufs=2))
    cprob = ctx.enter_context(tc.tile_pool(name="cprob", bufs=4))
    sprob = ctx.enter_context(tc.tile_pool(name="sprob", bufs=4))
    maskp = ctx.enter_context(tc.tile_pool(name="maskp", bufs=3))
    smallp = ctx.enter_context(tc.tile_pool(name="smallp", bufs=2))
    osb = ctx.enter_context(tc.tile_pool(name="osb", bufs=2))
    outp = ctx.enter_context(tc.tile_pool(name="outp", bufs=3))
    pp_qk = ctx.enter_context(tc.tile_pool(name="pp_qk", bufs=2, space="PSUM"))
    pp_misc = ctx.enter_context(tc.tile_pool(name="pp_misc", bufs=2, space="PSUM"))
    pp_pv = ctx.enter_context(tc.tile_pool(name="pp_pv", bufs=1, space="PSUM"))
    dramp = ctx.enter_context(tc.tile_pool(name="dramp", bufs=2, space="DRAM"))

    # ---- load constants ----
    kT_sb = const.tile([128, T], F32, tag="kT")
    nc.sync.dma_start(out=kT_sb, in_=d["kT"])
    vext_sb = const.tile([128, NT, 129], F32, tag="vext")
    nc.sync.dma_start(out=vext_sb, in_=d["v_ext"])
    cmpkT_sb = const.tile([128, M], F32, tag="cmpkT")
    nc.sync.dma_start(out=cmpkT_sb, in_=d["cmp_kT"])
    cmprhs_sb = const.tile([M, 161], F32, tag="cmprhs")
    nc.sync.dma_start(out=cmprhs_sb, in_=d["cmp_rhs"])
    cmpkeepT_sb = const.tile([M, NT, 128], F32, tag="cmpkeepT")
    nc.sync.dma_start(out=cmpkeepT_sb, in_=d["cmp_keepT"])
    slckeep_sb = const.tile([128, NT, NB], F32, tag="slckeep")
    nc.sync.dma_start(out=slckeep_sb, in_=d["slc_keep"])
    slcovr_sb = const.tile([128, NT, NB], F32, tag="slcovr")
    nc.sync.dma_start(out=slcovr_sb, in_=d["slc_ovr"])
    tri_i_sb = const.tile([128, 128], F32, tag="tri_i")
    nc.sync.dma_start(out=tri_i_sb, in_=d["tri_i"])
    tri_s_sb = const.tile([128, 128], F32, tag="tri_s")
    nc.sync.dma_start(out=tri_s_sb, in_=d["tri_s"])
    ident_sb = const.tile([128, 128], F32, tag="ident")
    nc.sync.dma_start(out=ident_sb, in_=d["ident"])
    cw_sb = const.tile([128, NT, 3 * HPC], F32, tag="cw")
    nc.sync.dma_start(out=cw_sb, in_=d["cw_own"])

    for qt in range(NT):
        # ---- per-tile loads ----
        qt_sb = qt_pool.tile([128, HQ, 128], F32, tag="qt")
        nc.sync.dma_start(out=qt_sb, in_=d["qt_all"][qt])
        qo_sb = qo_pool.tile([128, HPC, 128], F32, tag="qo")
        nc.sync.dma_start(out=qo_sb, in_=d["q_own"][qt])

        # ---- compressed attention, all heads: build slc_acc ----
        slc_acc = smallp.tile([128, NB], F32, tag="slc_acc")
        for h in range(HQ):
            ps_c = pp_qk.tile([M, 128], F32, tag="qk")
            nc.tensor.matmul(ps_c, lhsT=cmpkT_sb, rhs=qt_sb[:, h, :],
                             start=True, stop=True)
            pT = cprob.tile([M, 128], F32, tag="cp")
            nc.scalar.activation(pT, ps_c, EXP, scale=SCALE)
            nc.vector.tensor_mul(pT, pT, cmpkeepT_sb[:, qt, :])
            ps_s = pp_misc.tile([128, 33], F32, tag="misc")
            nc.tensor.matmul(ps_s, lhsT=pT, rhs=cmprhs_sb[:, 0:33],
                             start=True, stop=True)
            dn = smallp.tile([128, 1], F32, tag="dn")
            nc.vector.tensor_scalar(dn, ps_s[:, 32:33], EPS, None, op0=ADD)
            rc = smallp.tile([128, 1], F32, tag="rc")
            nc.vector.reciprocal(rc, dn)
            if h == 0:
                nc.vector.tensor_scalar(slc_acc, ps_s[:, 0:32], rc, None, op0=MUL)
            else:
                t_s = smallp.tile([128, NB], F32, tag="t_s")
                nc.vector.tensor_scalar(t_s, ps_s[:, 0:32], rc, None, op0=MUL)
                nc.vector.tensor_add(slc_acc, slc_acc, t_s)

        # ---- compressed attention, own heads: cmp_o ----
        cmp_o = []
        for oh in range(HPC):
            ps_c = pp_qk.tile([M, 128], F32, tag="qk")
            nc.tensor.matmul(ps_c, lhsT=cmpkT_sb, rhs=qo_sb[:, oh, :],
                             start=True, stop=True)
            pT = cprob.tile([M, 128], F32, tag="cp")
            nc.scalar.activation(pT, ps_c, EXP, scale=SCALE)
            nc.vector.tensor_mul(pT, pT, cmpkeepT_sb[:, qt, :])
            ps_o = pp_misc.tile([128, 129], F32, tag="misc")
            nc.tensor.matmul(ps_o, lhsT=pT, rhs=cmprhs_sb[:, 32:161],
                             start=True, stop=True)
            dn = smallp.tile([128, 1], F32, tag="dn")
            nc.vector.tensor_scalar(dn, ps_o[:, 0:1], EPS, None, op0=ADD)
            rc = smallp.tile([128, 1], F32, tag="rc")
            nc.vector.reciprocal(rc, dn)
            co = osb.tile([128, 128], F32, tag=f"cmp_o{oh}")
            nc.vector.tensor_scalar(co, ps_o[:, 1:129], rc, None, op0=MUL)
            cmp_o.append(co)

        # ---- top-k block mask ----
        slc_fin = smallp.tile([128, NB], F32, tag="slc_fin")
        nc.vector.tensor_mul(slc_fin, slc_acc, slckeep_sb[:, qt, :])
        nc.vector.tensor_add(slc_fin, slc_fin, slcovr_sb[:, qt, :])
        z1 = smallp.tile([128, NB], F32, tag="z1")
        mx = smallp.tile([128, 8], F32, tag="mx")
        nc.vector.max(mx, slc_fin)
        nc.vector.match_replace(z1, mx, slc_fin, ZAP)
        z2 = smallp.tile([128, NB], F32, tag="z2")
        mx2 = smallp.tile([128, 8], F32, tag="mx2")
        nc.vector.max(mx2, z1)
        nc.vector.match_replace(z2, mx2, z1, ZAP)
        keep01 = smallp.tile([128, NB], F32, tag="keep01")
        nc.vector.tensor_tensor(keep01, slc_fin, z2, op=ISEQ)   # 1 = NOT selected
        nc.vector.tensor_scalar(keep01, keep01, -1.0, 1.0, op0=MUL, op1=ADD)
        ps_kt = pp_misc.tile([NB, 128], F32, tag="misc")
        nc.tensor.transpose(ps_kt, keep01, ident_sb)
        keepT_sb = smallp.tile([NB, 128], F32, tag="keepT_sb")
        nc.scalar.activation(keepT_sb, ps_kt, CPY)
        keepT = dramp.tile([NB, 128], F32, tag="keepT")
        nc.sync.dma_start(out=keepT, in_=keepT_sb)

        # ---- selected-block attention (own heads) ----
        ps_sel = [pp_pv.tile([128, 129], F32, tag=f"sel{oh}", name=f"ps_sel{oh}")
                  for oh in range(HPC)]
        for c in range(qt + 1):
            mt = maskp.tile([128, 128], F32, tag="mt")
            nc.gpsimd.dma_start(out=mt[0:64, :],
                                in_=keepT[2 * c].partition_broadcast(64))
            nc.gpsimd.dma_start(out=mt[64:128, :],
                                in_=keepT[2 * c + 1].partition_broadcast(64))
            if c == qt:
                nc.vector.tensor_mul(mt, mt, tri_i_sb)
            for oh in range(HPC):
                ps_qk_t = pp_qk.tile([128, 128], F32, tag="qk")
                nc.tensor.matmul(ps_qk_t, lhsT=kT_sb[:, c * 128:(c + 1) * 128],
                                 rhs=qo_sb[:, oh, :], start=True, stop=True)
                pT = sprob.tile([128, 128], F32, tag="sp")
                nc.scalar.activation(pT, ps_qk_t, EXP, scale=SCALE)
                nc.vector.tensor_mul(pT, pT, mt)
                nc.tensor.matmul(ps_sel[oh], lhsT=pT, rhs=vext_sb[:, c, :],
                                 start=(c == 0), stop=(c == qt))
        slc_o = []
        for oh in range(HPC):
            dn = smallp.tile([128, 1], F32, tag="dn")
            nc.vector.tensor_scalar(dn, ps_sel[oh][:, 128:129], EPS, None, op0=ADD)
            rc = smallp.tile([128, 1], F32, tag="rc")
            nc.vector.reciprocal(rc, dn)
            so = osb.tile([128, 128], F32, tag=f"slc_o{oh}")
            nc.vector.tensor_scalar(so, ps_sel[oh][:, 0:128], rc, None, op0=MUL)
            slc_o.append(so)

        # ---- sliding-window attention (own heads) ----
        c_lo = max(0, qt - WCH)
        ps_swa = [pp_pv.tile([128, 129], F32, tag=f"swa{oh}", name=f"ps_swa{oh}")
                  for oh in range(HPC)]
        for c in range(c_lo, qt + 1):
            for oh in range(HPC):
                ps_qk_t = pp_qk.tile([128, 128], F32, tag="qk")
                nc.tensor.matmul(ps_qk_t, lhsT=kT_sb[:, c * 128:(c + 1) * 128],
                                 rhs=qo_sb[:, oh, :], start=True, stop=True)
                pT = sprob.tile([128, 128], F32, tag="sp")
                nc.scalar.activation(pT, ps_qk_t, EXP, scale=SCALE)
                if c == qt:
                    nc.vector.tensor_mul(pT, pT, tri_i_sb)
                elif c == qt - WCH:
                    nc.vector.tensor_mul(pT, pT, tri_s_sb)
                nc.tensor.matmul(ps_swa[oh], lhsT=pT, rhs=vext_sb[:, c, :],
                                 start=(c == c_lo), stop=(c == qt))
        swa_o = []
        for oh in range(HPC):
            dn = smallp.tile([128, 1], F32, tag="dn")
            nc.vector.tensor_scalar(dn, ps_swa[oh][:, 128:129], EPS, None, op0=ADD)
            rc = smallp.tile([128, 1], F32, tag="rc")
            nc.vector.reciprocal(rc, dn)
            wo = osb.tile([128, 128], F32, tag=f"swa_o{oh}")
            nc.vector.tensor_scalar(wo, ps_swa[oh][:, 0:128], rc, None, op0=MUL)
            swa_o.append(wo)

        # ---- combine + store ----
        sg = smallp.tile([128, 3 * HPC], F32, tag="sg")
        nc.scalar.activation(sg, cw_sb[:, qt, :], SIG)
        for oh in range(HPC):
            ot = outp.tile([128, 128], F32, tag="ot")
            nc.vector.tensor_scalar(ot, cmp_o[oh], sg[:, 3 * oh:3 * oh + 1],
                                    None, op0=MUL)
            tmp = outp.tile([128, 128], F32, tag="tmp")
            nc.vector.tensor_scalar(tmp, slc_o[oh], sg[:, 3 * oh + 1:3 * oh + 2],
                                    None, op0=MUL)
            nc.vector.tensor_add(ot, ot, tmp)
            tmp2 = outp.tile([128, 128], F32, tag="tmp")
            nc.vector.tensor_scalar(tmp2, swa_o[oh], sg[:, 3 * oh + 2:3 * oh + 3],
                                    None, op0=MUL)
            nc.vector.tensor_add(ot, ot, tmp2)
            nc.sync.dma_start(out=out_ap[qt * 128:(qt + 1) * 128, oh, :], in_=ot)


def _host_inputs(q, k, v, cw):
    """Precompute per-core input arrays (numpy only; no FLOP-heavy work)."""
    q = np.ascontiguousarray(q, np.float32)
    k2 = np.ascontiguousarray(k[:, 0, :], np.float32)
    v2 = np.ascontiguousarray(v[:, 0, :], np.float32)
    cw = np.ascontiguousarray(cw, np.float32)

    # [tile, d, h, q]
    qt_all = np.ascontiguousarray(
        q.reshape(NT, 128, HQ, D).transpose(0, 3, 2, 1))
    kT = np.ascontiguousarray(k2.T)
    v_ext = np.ascontiguousarray(
        np.concatenate([v2, np.ones((T, 1), np.float32)], 1)
        .reshape(NT, 128, 129).transpose(1, 0, 2))
    idx = np.arange(M)[:, None] * ST + np.arange(KS)[None, :]
    cmp_k = k2[idx].mean(1)
    cmp_v = v2[idx].mean(1)
    cmp_kT = np.ascontiguousarray(cmp_k.T)
    ju, bu = KS // ST, BS // ST
    j = np.arange(M)[:, None]
    b = np.arange(NB)[None, :]
    ov = np.maximum(0, np.minimum(j + ju, (b + 1) * bu) - np.maximum(j, b * bu))
    Wmap = (ov / ju).astype(np.float32)
    cmp_rhs = np.ascontiguousarray(
        np.concatenate([Wmap, np.ones((M, 1), np.float32), cmp_v], 1))
    t_pos = np.arange(T)
    cmp_keepT = np.ascontiguousarray(
        ((np.arange(M)[:, None] * ST + KS) <= (t_pos[None, :] + 1))
        .astype(np.float32).reshape(M, NT, 128))
    cur_blk = t_pos // BS
    bidx = np.arange(NB)
    future = bidx[None, :] > cur_blk[:, None]
    forced = (bidx[None, :] < NINIT) | (
        (bidx[None, :] <= cur_blk[:, None])
        & (bidx[None, :] > cur_blk[:, None] - NLOCAL))
    slc_keep = np.ascontiguousarray(
        (~(future | forced)).astype(np.float32)
        .reshape(NT, 128, NB).transpose(1, 0, 2))
    slc_ovr = np.ascontiguousarray(
        np.where(forced, BIGC, np.where(future, NEGC, 0.0)).astype(np.float32)
        .reshape(NT, 128, NB).transpose(1, 0, 2))
    ar = np.arange(128)
    tri_i = (ar[None, :] >= ar[:, None]).astype(np.float32)
    tri_s = (ar[None, :] < ar[:, None]).astype(np.float32)
    ident = np.eye(128, dtype=np.float32)

    shared = dict(qt_all=qt_all, kT=kT, v_ext=v_ext, cmp_kT=cmp_kT,
                  cmp_rhs=cmp_rhs, cmp_keepT=cmp_keepT, slc_keep=slc_keep,
                  slc_ovr=slc_ovr, tri_i=tri_i, tri_s=tri_s, ident=ident)
    in_maps = []
    for c in range(NCORES):
        hs = slice(c * HPC, (c + 1) * HPC)
        q_own = np.ascontiguousarray(
            q[:, hs, :].reshape(NT, 128, HPC, D).transpose(0, 3, 2, 1))
        cw_own = np.ascontiguousarray(
            cw[:, hs, :].reshape(NT, 128, 3 * HPC).transpose(1, 0, 2))
        in_maps.append(dict(shared, q_own=q_own, cw_own=cw_own))
    return in_maps


_PROGRAM = None


def _get_program():
    global _PROGRAM
    if _PROGRAM is None:
        _PROGRAM = _build_program()
    return _PROGRAM


def kernel(q, k, v, combine_weight, cu_seqlens, _trace=False):
    nc = _get_program()
    in_maps = _host_inputs(np.asarray(q), np.asarray(k), np.asarray(v),
                           np.asarray(combine_weight))
    res = run_bass_kernel_spmd(nc, in_maps, core_ids=list(range(NCORES)),
                               trace=_trace)
    outs = [res.results[c]["out"] for c in range(NCORES)]
    full = np.concatenate(outs, axis=1).astype(np.float32)
    if _trace:
        kernel._last_results = res
    return full

